# revision 2
# baseline (speedup 1.0000x reference)
"""Trainium2 Bass kernel for causal self-attention with RoPE and tanh scoring.

Reference computation (per batch b, head h):
    q,k = rope(split_heads(Q)), rope(split_heads(K)); v = split_heads(V)
    scores = q @ k^T / sqrt(hs);  att = tanh(where(causal, scores, -inf))
    (masked positions become tanh(-inf) = -1 and DO contribute -1 * v)
    out = att @ v
Sharding: 32 (b,h) pairs -> 4 per core across 8 cores.

All device data is bf16 (inputs, RoPE consts, att weights); matmuls
accumulate in fp32 PSUM and the output is written fp32. This halves HBM
traffic vs fp32 and keeps every matmul at the PE's 1 cycle/row rate.
See kernel docstring history: scoresT formulation, diagonal-band mask
via accumulating one-hot matmuls, fully-masked tiles folded into a
correction term corr = -sum of v rows beyond the chunk.
"""

import sys

if "/opt/trn_rl_repo" not in sys.path:
    sys.path.insert(0, "/opt/trn_rl_repo")

import numpy as np

B, T, C_EMB = 2, 2048, 2048
NH, HS = 16, 128
NCORES = 8
PAIRS = (B * NH) // NCORES  # 4 (b,h) pairs per core
NQ = 512                    # q-chunk width (PSUM bank = 512 fp32)
NKT = 128                   # k-tile rows
JT = T // NKT               # 16 k-tiles
NCH = T // NQ               # 4 q-chunks
BIG = 1.0e4
SCALE = 1.0 / np.sqrt(HS)

def _host_consts():
    """Per-core constant tensors (identical on every core)."""
    import ml_dtypes
    BF = ml_dtypes.bfloat16
    i = np.arange(HS // 2, dtype=np.float64)
    freqs = 1.0 / 10000.0 ** (2.0 * i / HS)           # [64]
    t = np.arange(T, dtype=np.float64)
    ang = np.outer(freqs, t)                           # [64, T]
    cos = np.cos(ang)
    sin = np.sin(ang)
    rope_c = np.concatenate([cos, cos], axis=0).astype(BF)    # [128, T]
    rope_s = np.concatenate([-sin, sin], axis=0).astype(BF)   # [128, T]

    # mask lhsT[k, p] = -BIG if k <= p (upper triangular incl diag)
    mask_a = (-BIG * np.triu(np.ones((NKT, NKT)))).astype(BF)

    # mask rhs one-hot tiles, one per diagonal-band alignment r = J - 4C
    # S^T[p, f] needs -BIG iff (128J + p) > (512C + f)  <=>  p >= f - 128r + 1
    mask_b = np.zeros((NKT, NCH, NQ), np.float32)
    for r in range(NCH):
        for f in range(NQ):
            th = f - NKT * r + 1
            if th <= NKT - 1:
                mask_b[max(th, 0), r, f] = 1.0

    # step mask for corrections: SM[p, J, c] = -1 if (128J + p) >= 512(c+1)
    sm = np.zeros((NKT, JT, NCH), np.float32)
    for j in range(JT):
        for c in range(NCH):
            tk = j * NKT + np.arange(NKT)
            sm[:, j, c] = np.where(tk >= NQ * (c + 1), -1.0, 0.0)

    return {"rope_c": rope_c, "rope_s": rope_s,
            "mask_a": mask_a, "mask_b": mask_b.astype(BF),
            "sm": sm.astype(BF)}


def _build_program(reps=1):
    import concourse.bacc as bacc
    import concourse.mybir as mybir
    import concourse.tile as tile

    F32 = mybir.dt.float32
    BF16 = mybir.dt.bfloat16
    AFT = mybir.ActivationFunctionType

    nc = bacc.Bacc("TRN2", target_bir_lowering=False, debug=False)

    qk_d = nc.dram_tensor("qkT", [PAIRS, 2, HS, T], BF16, kind="ExternalInput")
    v_d = nc.dram_tensor("v", [PAIRS, NKT, JT, HS], BF16, kind="ExternalInput")
    rc_d = nc.dram_tensor("rope_c", [HS, T], BF16, kind="ExternalInput")
    rs_d = nc.dram_tensor("rope_s", [HS, T], BF16, kind="ExternalInput")
    ma_d = nc.dram_tensor("mask_a", [NKT, NKT], BF16, kind="ExternalInput")
    mb_d = nc.dram_tensor("mask_b", [NKT, NCH, NQ], BF16, kind="ExternalInput")
    sm_d = nc.dram_tensor("sm", [NKT, JT, NCH], BF16, kind="ExternalInput")
    out_d = nc.dram_tensor("outT", [PAIRS, HS, T], F32, kind="ExternalOutput")

    with tile.TileContext(nc) as tc:
        with (
            tc.tile_pool(name="consts", bufs=1) as consts,
            tc.tile_pool(name="qc", bufs=9) as q_pool,
            tc.tile_pool(name="vp", bufs=8) as v_pool,
            tc.tile_pool(name="ropet", bufs=3) as t_pool,
            tc.tile_pool(name="att", bufs=5) as att_pool,
            tc.tile_pool(name="osb", bufs=3) as osb_pool,
            tc.tile_pool(name="corr", bufs=2) as corr_pool,
            tc.tile_pool(name="psS", bufs=3, space="PSUM") as psS,
            tc.tile_pool(name="psO", bufs=2, space="PSUM") as psO,
        ):
            rc = consts.tile([HS, T], BF16)
            rs = consts.tile([HS, T], BF16)
            ma = consts.tile([NKT, NKT], BF16)
            mb = consts.tile([NKT, NCH, NQ], BF16)
            sm = consts.tile([NKT, JT, NCH], BF16)
            nc.scalar.dma_start(out=ma, in_=ma_d.ap())
            nc.scalar.dma_start(out=mb, in_=mb_d.ap())
            nc.scalar.dma_start(out=sm, in_=sm_d.ap())
            for h0, h1 in ((0, T // 2), (T // 2, T)):
                nc.scalar.dma_start(out=rc[:, h0:h1], in_=rc_d.ap()[:, h0:h1])
                nc.scalar.dma_start(out=rs[:, h0:h1], in_=rs_d.ap()[:, h0:h1])

            import concourse.bass as bass

            def _bcast2(ap):
                """[HS, NQ] slice -> [HS, 2, NQ] with a 0-stride middle dim."""
                return bass.AP(tensor=ap.tensor, offset=ap.offset,
                               ap=[list(ap.ap[0]), [0, 2], list(ap.ap[1])])

            def _load_rope_chunk(g, ch):
                """Load a [HS, 2, NQ] q+k column chunk and apply RoPE."""
                sl = slice(ch * NQ, (ch + 1) * NQ)
                src = qk_d.ap()[g].rearrange("s p t -> p s t")   # [HS, 2, T]
                x = q_pool.tile([HS, 2, NQ], BF16, tag="qk")
                nc.sync.dma_start(out=x, in_=src[:, :, sl])
                # partition-rotated re-read of the same DRAM chunk:
                # rows 64..127 land on partitions 0..63 and vice versa
                xs = t_pool.tile([HS, 2, NQ], BF16, tag="xs")
                nc.sync.dma_start(out=xs[0:64], in_=src[64:128, :, sl])
                nc.sync.dma_start(out=xs[64:128], in_=src[0:64, :, sl])
                t1 = t_pool.tile([HS, 2, NQ], BF16, tag="t1")
                t2 = t_pool.tile([HS, 2, NQ], BF16, tag="t2")
                nc.gpsimd.tensor_mul(t1, x, _bcast2(rc[:, sl]))
                nc.vector.tensor_mul(t2, xs, _bcast2(rs[:, sl]))
                nc.vector.tensor_add(x, t1, t2)
                return x

            def _load_pair(g):
                kch, qch, vq = [], [], []
                v_src = v_d.ap()[g]  # [NKT, JT, HS], contiguous per partition
                for ch in range(NCH):
                    x = _load_rope_chunk(g, ch)
                    qch.append(x[:, 0, :])
                    kch.append(x[:, 1, :])
                    vt = v_pool.tile([NKT, 4, HS], BF16, tag="v")
                    nc.sync.dma_start(out=vt, in_=v_src[:, 4 * ch:4 * ch + 4, :])
                    vq.append(vt)
                return kch, qch, vq

            def _one_pair(g, loaded, nxt):
                kch, qch, vq = loaded

                def v_of(j):
                    return vq[j // 4][:, j % 4, :]

                corr_sb = corr_pool.tile([HS, NCH], F32)

                def _emit_corr():
                    # corr[d, c] = -sum_{tk >= 512(c+1)} v[tk, d]
                    corr_ps = psO.tile([HS, NCH], mybir.dt.float32, tag="o")
                    for j in range(JT):
                        nc.tensor.matmul(corr_ps, v_of(j), sm[:, j, :],
                                         start=(j == 0), stop=(j == JT - 1))
                    nc.vector.tensor_copy(corr_sb, corr_ps)

                # ---- attention ----
                from collections import deque
                pending = deque()  # software pipeline: AV lags two groups

                def _emit_av(item):
                    o_ps, att, ja, jb, last, c = item
                    nc.tensor.matmul(o_ps, v_of(ja), att[:, 0, :],
                                     start=(ja == 0), stop=False)
                    nc.tensor.matmul(o_ps, v_of(jb), att[:, 1, :],
                                     start=False, stop=last)
                    if last:
                        # copy the finished chunk out right away so its
                        # PSUM bank frees as early as possible
                        o_sb = osb_pool.tile([HS, NQ], F32)
                        nc.vector.tensor_scalar_add(o_sb, o_ps,
                                                    corr_sb[:, c:c + 1])
                        nc.sync.dma_start(
                            out=out_d.ap()[g][:, c * NQ:(c + 1) * NQ],
                            in_=o_sb)

                for c in range(NCH):
                    n_j = 4 * c + 4  # k-tiles 0..4c+3
                    o_ps = psO.tile([HS, NQ], mybir.dt.float32, tag="o")
                    for jp in range(n_j // 2):
                        ja, jb = 2 * jp, 2 * jp + 1
                        # Band tiles (j >= 4c) need masking; the pair shares a
                        # 256-aligned compute window [off, NQ).
                        r_a, r_b = ja - 4 * c, jb - 4 * c
                        off = 256 if r_a >= 2 else 0
                        s = psS.tile([NKT, 2, NQ], mybir.dt.float32, tag="s")
                        for idx, j, r in ((0, ja, r_a), (1, jb, r_b)):
                            nc.tensor.matmul(
                                s[:, idx, off:],
                                kch[j // 4][:, (j % 4) * NKT:(j % 4 + 1) * NKT],
                                qch[c][:, off:],
                                start=True, stop=not (r >= 0))
                            if r >= 0:
                                # add -BIG above the causal boundary across
                                # [off, 128r+128) — everything left of the
                                # compute window is memset to -1 instead
                                moff = off
                                mn = 128 * r + 128 - moff
                                nc.tensor.matmul(
                                    s[:, idx, moff:moff + mn], ma,
                                    mb[:, r, moff:moff + mn],
                                    start=False, stop=True)
                        att = att_pool.tile([NKT, 2, NQ], BF16)
                        if off:
                            nc.gpsimd.memset(att[:, :, 0:off], -1.0)
                        nc.scalar.activation(att[:, :, off:], s[:, :, off:],
                                             AFT.Tanh, scale=float(SCALE))
                        pending.append((o_ps, att, ja, jb, jb == n_j - 1, c))
                        if len(pending) > 2:
                            _emit_av(pending.popleft())
                    if c == 0:
                        _emit_corr()
                        if nxt is not None:
                            # emit next pair's loads/RoPE so DMA/Pool/DVE
                            # fill them in during this pair's attention
                            nxt.append(_load_pair(g + 1))
                while pending:
                    _emit_av(pending.popleft())

            def _pairs_body():
                loaded = _load_pair(0)
                for g in range(PAIRS):
                    nxt = [] if g + 1 < PAIRS else None
                    _one_pair(g, loaded, nxt)
                    loaded = nxt[0] if nxt else None

            if reps == 1:
                _pairs_body()
            else:
                with tc.For_i(0, reps, 1,
                              hint_engines=(mybir.EngineType.PE,
                                            mybir.EngineType.Activation,
                                            mybir.EngineType.SP)):
                    _pairs_body()

    nc.compile()
    return nc


_PROGRAMS = {}


def _get_program(reps=1):
    if reps not in _PROGRAMS:
        _PROGRAMS[reps] = _build_program(reps)
    return _PROGRAMS[reps]


def _shard_inputs(Q, K, V):
    import ml_dtypes
    BF = ml_dtypes.bfloat16
    consts = _host_consts()
    d = np.arange(HS)
    perm = np.concatenate([d[0::2], d[1::2]])  # deinterleave head dim

    in_maps = []
    for core in range(NCORES):
        qkT = np.empty((PAIRS, 2, HS, T), BF)
        v = np.empty((PAIRS, NKT, JT, HS), BF)
        for slot in range(PAIRS):
            g = core * PAIRS + slot
            b, h = divmod(g, NH)
            cols = h * HS + np.arange(HS)
            qkT[slot, 0] = Q[b][:, cols[perm]].T.astype(BF)
            qkT[slot, 1] = K[b][:, cols[perm]].T.astype(BF)
            v[slot] = V[b][:, cols].reshape(JT, NKT, HS).transpose(1, 0, 2).astype(BF)
        in_maps.append({
            "qkT": np.ascontiguousarray(qkT),
            "v": np.ascontiguousarray(v),
            "rope_c": consts["rope_c"],
            "rope_s": consts["rope_s"],
            "mask_a": consts["mask_a"],
            "mask_b": consts["mask_b"],
            "sm": consts["sm"],
        })
    return in_maps


def _gather_outputs(per_core_outT):
    out = np.empty((B, T, C_EMB), np.float32)
    for core in range(NCORES):
        outT = per_core_outT[core]  # [PAIRS, HS, T]
        for slot in range(PAIRS):
            g = core * PAIRS + slot
            b, h = divmod(g, NH)
            out[b, :, h * HS:(h + 1) * HS] = outT[slot].T
    return out


def kernel(Q, K, V):
    from concourse.bass_utils import run_bass_kernel_spmd

    Q = np.asarray(Q, dtype=np.float32)
    K = np.asarray(K, dtype=np.float32)
    V = np.asarray(V, dtype=np.float32)

    nc = _get_program()
    in_maps = _shard_inputs(Q, K, V)
    res = run_bass_kernel_spmd(nc, in_maps, core_ids=list(range(NCORES)))
    return _gather_outputs([res.results[c]["outT"] for c in range(NCORES)])


# revision 3
# speedup vs baseline: 2.2699x; 2.2699x over previous
"""Trainium2 Bass kernel for causal self-attention with RoPE and tanh scoring.

Reference computation (per batch b, head h):
    q,k = rope(split_heads(Q)), rope(split_heads(K)); v = split_heads(V)
    scores = q @ k^T / sqrt(hs);  att = tanh(where(causal, scores, -inf))
    (masked positions become tanh(-inf) = -1 and DO contribute -1 * v)
    out = att @ v
Sharding: 32 (b,h) pairs -> 4 per core across 8 cores.

All device data is bf16; matmuls accumulate in fp32 PSUM; output fp32.
S^T formulation (scoresT[tk, tq]) per 512-wide q-chunk over the lower
triangle of k-tiles only. Diagonal-band masking is done AFTER tanh with
a DVE min against a +-1 triangular mask on the 128-wide diagonal
windows (tanh(s) <= 1, so min(tanh, -1) = -1 exactly on masked cells);
fully-masked 128-col strips are memset to -1; fully-masked k-tiles are
folded into corr = -sum of v rows beyond the chunk (tiny step-mask
matmuls).
"""

import sys

if "/opt/trn_rl_repo" not in sys.path:
    sys.path.insert(0, "/opt/trn_rl_repo")

import numpy as np

B, T, C_EMB = 2, 2048, 2048
NH, HS = 16, 128
NCORES = 8
PAIRS = (B * NH) // NCORES  # 4 (b,h) pairs per core
NQ = 512                    # q-chunk width (PSUM bank = 512 fp32)
NKT = 128                   # k-tile rows
JT = T // NKT               # 16 k-tiles
NCH = T // NQ               # 4 q-chunks
SCALE = 1.0 / np.sqrt(HS)

def _host_consts():
    """Per-core constant tensors (identical on every core)."""
    import ml_dtypes
    BF = ml_dtypes.bfloat16
    i = np.arange(HS // 2, dtype=np.float64)
    freqs = 1.0 / 10000.0 ** (2.0 * i / HS)           # [64]
    t = np.arange(T, dtype=np.float64)
    ang = np.outer(freqs, t)                           # [64, T]
    cos = np.cos(ang)
    sin = np.sin(ang)
    rope_c = np.concatenate([cos, cos], axis=0).astype(BF)    # [128, T]
    rope_s = np.concatenate([-sin, sin], axis=0).astype(BF)   # [128, T]

    # diagonal-window mask: S^T[p, f] masked iff tk > tq <=> p > j within
    # the 128-wide diagonal sub-block (j = f - 128r)
    pj = np.arange(NKT)
    dmin = np.where(pj[:, None] > pj[None, :], -1.0, 1.0).astype(BF)

    # step mask for corrections: SM[p, J, c] = -1 if (128J + p) >= 512(c+1)
    sm = np.zeros((NKT, JT, NCH), np.float32)
    for j in range(JT):
        for c in range(NCH):
            tk = j * NKT + np.arange(NKT)
            sm[:, j, c] = np.where(tk >= NQ * (c + 1), -1.0, 0.0)

    return {"rope_c": rope_c, "rope_s": rope_s,
            "dmin": dmin, "sm": sm.astype(BF)}


def _build_program(reps=1):
    import concourse.bacc as bacc
    import concourse.mybir as mybir
    import concourse.tile as tile

    F32 = mybir.dt.float32
    BF16 = mybir.dt.bfloat16
    AFT = mybir.ActivationFunctionType

    nc = bacc.Bacc("TRN2", target_bir_lowering=False, debug=False)

    qk_d = nc.dram_tensor("qkT", [PAIRS, 2, HS, T], BF16, kind="ExternalInput")
    v_d = nc.dram_tensor("v", [PAIRS, NKT, JT, HS], BF16, kind="ExternalInput")
    rc_d = nc.dram_tensor("rope_c", [HS, T], BF16, kind="ExternalInput")
    rs_d = nc.dram_tensor("rope_s", [HS, T], BF16, kind="ExternalInput")
    dm_d = nc.dram_tensor("dmin", [NKT, NKT], BF16, kind="ExternalInput")
    sm_d = nc.dram_tensor("sm", [NKT, JT, NCH], BF16, kind="ExternalInput")
    out_d = nc.dram_tensor("outT", [PAIRS, HS, T], F32, kind="ExternalOutput")

    with tile.TileContext(nc) as tc:
        with (
            tc.tile_pool(name="consts", bufs=1) as consts,
            tc.tile_pool(name="qc", bufs=9) as q_pool,
            tc.tile_pool(name="vp", bufs=8) as v_pool,
            tc.tile_pool(name="ropet", bufs=3) as t_pool,
            tc.tile_pool(name="att", bufs=5) as att_pool,
            tc.tile_pool(name="osb", bufs=3) as osb_pool,
            tc.tile_pool(name="corr", bufs=2) as corr_pool,
            tc.tile_pool(name="psS", bufs=3, space="PSUM") as psS,
            tc.tile_pool(name="psO", bufs=2, space="PSUM") as psO,
        ):
            rc = consts.tile([HS, T], BF16)
            rs = consts.tile([HS, T], BF16)
            dm = consts.tile([NKT, NKT], BF16)
            sm = consts.tile([NKT, JT, NCH], BF16)
            nc.scalar.dma_start(out=dm, in_=dm_d.ap())
            nc.scalar.dma_start(out=sm, in_=sm_d.ap())
            # chunked so chunk-0 RoPE doesn't wait on the full table
            for ch in range(NCH):
                h0, h1 = ch * NQ, (ch + 1) * NQ
                nc.scalar.dma_start(out=rc[:, h0:h1], in_=rc_d.ap()[:, h0:h1])
                nc.scalar.dma_start(out=rs[:, h0:h1], in_=rs_d.ap()[:, h0:h1])

            import concourse.bass as bass

            def _bcast2(ap):
                """[HS, NQ] slice -> [HS, 2, NQ] with a 0-stride middle dim."""
                return bass.AP(tensor=ap.tensor, offset=ap.offset,
                               ap=[list(ap.ap[0]), [0, 2], list(ap.ap[1])])

            def _diagwin(a, r0):
                """att [NKT, 2, NQ] -> [NKT, 2, NKT] windows at cols
                128*r0 (idx 0) and 128*(r0+1) (idx 1): mid-stride trick."""
                return bass.AP(tensor=a.tensor, offset=a.offset + NKT * r0,
                               ap=[list(a.ap[0]),
                                   [a.ap[1][0] + NKT, 2], [1, NKT]])

            def _load_rope_chunk(g, ch):
                """Load a [HS, 2, NQ] q+k column chunk and apply RoPE."""
                sl = slice(ch * NQ, (ch + 1) * NQ)
                src = qk_d.ap()[g].rearrange("s p t -> p s t")   # [HS, 2, T]
                x = q_pool.tile([HS, 2, NQ], BF16, tag="qk")
                nc.sync.dma_start(out=x, in_=src[:, :, sl])
                # partition-rotated re-read of the same DRAM chunk:
                # rows 64..127 land on partitions 0..63 and vice versa
                xs = t_pool.tile([HS, 2, NQ], BF16, tag="xs")
                nc.sync.dma_start(out=xs[0:64], in_=src[64:128, :, sl])
                nc.sync.dma_start(out=xs[64:128], in_=src[0:64, :, sl])
                t1 = t_pool.tile([HS, 2, NQ], BF16, tag="t1")
                t2 = t_pool.tile([HS, 2, NQ], BF16, tag="t2")
                nc.vector.tensor_mul(t1, x, _bcast2(rc[:, sl]))
                nc.vector.tensor_mul(t2, xs, _bcast2(rs[:, sl]))
                nc.vector.tensor_add(x, t1, t2)
                return x

            def _load_pair(g):
                kch, qch, vq = [], [], []
                v_src = v_d.ap()[g]  # [NKT, JT, HS], contiguous per partition
                for ch in range(NCH):
                    x = _load_rope_chunk(g, ch)
                    qch.append(x[:, 0, :])
                    kch.append(x[:, 1, :])
                for ch in range(NCH):
                    vt = v_pool.tile([NKT, 4, HS], BF16, tag="v")
                    nc.sync.dma_start(out=vt, in_=v_src[:, 4 * ch:4 * ch + 4, :])
                    vq.append(vt)
                return kch, qch, vq

            def _one_pair(g, loaded, nxt):
                kch, qch, vq = loaded

                def v_of(j):
                    return vq[j // 4][:, j % 4, :]

                corr_sb = corr_pool.tile([HS, NCH], F32)

                def _emit_corr():
                    # corr[d, c] = -sum_{tk >= 512(c+1)} v[tk, d]
                    corr_ps = psO.tile([HS, NCH], mybir.dt.float32, tag="o")
                    for j in range(JT):
                        nc.tensor.matmul(corr_ps, v_of(j), sm[:, j, :],
                                         start=(j == 0), stop=(j == JT - 1))
                    nc.vector.tensor_copy(corr_sb, corr_ps)

                # ---- attention ----
                from collections import deque
                pending = deque()  # software pipeline: AV lags two groups

                def _emit_av(item):
                    o_ps, att, ja, jb, last, c = item
                    nc.tensor.matmul(o_ps, v_of(ja), att[:, 0, :],
                                     start=(ja == 0), stop=False)
                    nc.tensor.matmul(o_ps, v_of(jb), att[:, 1, :],
                                     start=False, stop=last)
                    if last:
                        # copy the finished chunk out right away so its
                        # PSUM bank frees as early as possible
                        o_sb = osb_pool.tile([HS, NQ], F32)
                        nc.gpsimd.tensor_scalar_add(o_sb, o_ps,
                                                    corr_sb[:, c:c + 1])
                        nc.sync.dma_start(
                            out=out_d.ap()[g][:, c * NQ:(c + 1) * NQ],
                            in_=o_sb)

                for c in range(NCH):
                    n_j = 4 * c + 4  # k-tiles 0..4c+3
                    o_ps = psO.tile([HS, NQ], mybir.dt.float32, tag="o")
                    last_ch = g == PAIRS - 1 and c == NCH - 1
                    for jp in range(n_j // 2):
                        ja, jb = 2 * jp, 2 * jp + 1
                        # Band tiles (j >= 4c) need masking; the pair shares a
                        # 256-aligned compute window [off, NQ).
                        r_a, r_b = ja - 4 * c, jb - 4 * c
                        off = 256 if r_a >= 2 else 0
                        s = psS.tile([NKT, 2, NQ], mybir.dt.float32, tag="s")
                        for idx, j in ((0, ja), (1, jb)):
                            nc.tensor.matmul(
                                s[:, idx, off:],
                                kch[j // 4][:, (j % 4) * NKT:(j % 4 + 1) * NKT],
                                qch[c][:, off:],
                                start=True, stop=True)
                        att = att_pool.tile([NKT, 2, NQ], BF16)
                        if off:
                            nc.gpsimd.memset(att[:, :, 0:off], -1.0)
                        nc.scalar.activation(att[:, :, off:], s[:, :, off:],
                                             AFT.Tanh, scale=float(SCALE))
                        if r_a >= 0:
                            # post-tanh causal mask on the two 128-wide
                            # diagonal windows: min(tanh, +-1 triangle)
                            nc.vector.tensor_tensor(
                                _diagwin(att, r_a), _diagwin(att, r_a),
                                _bcast2(dm[:, :]), mybir.AluOpType.min)
                            # subtile b's fully-masked strip left of its
                            # diagonal window
                            nc.gpsimd.memset(att[:, 1, off:off + NKT], -1.0)
                        pending.append((o_ps, att, ja, jb, jb == n_j - 1, c))
                        if len(pending) > (1 if last_ch else 2):
                            _emit_av(pending.popleft())
                    if c == 0:
                        _emit_corr()
                        if nxt is not None:
                            # emit next pair's loads/RoPE so DMA/Pool/DVE
                            # fill them in during this pair's attention
                            nxt.append(_load_pair(g + 1))
                while pending:
                    _emit_av(pending.popleft())

            def _pairs_body():
                loaded = _load_pair(0)
                for g in range(PAIRS):
                    nxt = [] if g + 1 < PAIRS else None
                    _one_pair(g, loaded, nxt)
                    loaded = nxt[0] if nxt else None

            if reps == 1:
                _pairs_body()
            else:
                with tc.For_i(0, reps, 1,
                              hint_engines=(mybir.EngineType.PE,
                                            mybir.EngineType.Activation,
                                            mybir.EngineType.SP)):
                    _pairs_body()

    nc.compile()
    return nc


_PROGRAMS = {}


def _get_program(reps=1):
    if reps not in _PROGRAMS:
        _PROGRAMS[reps] = _build_program(reps)
    return _PROGRAMS[reps]


def _shard_inputs(Q, K, V):
    import ml_dtypes
    BF = ml_dtypes.bfloat16
    consts = _host_consts()
    d = np.arange(HS)
    perm = np.concatenate([d[0::2], d[1::2]])  # deinterleave head dim

    in_maps = []
    for core in range(NCORES):
        qkT = np.empty((PAIRS, 2, HS, T), BF)
        v = np.empty((PAIRS, NKT, JT, HS), BF)
        for slot in range(PAIRS):
            g = core * PAIRS + slot
            b, h = divmod(g, NH)
            cols = h * HS + np.arange(HS)
            qkT[slot, 0] = Q[b][:, cols[perm]].T.astype(BF)
            qkT[slot, 1] = K[b][:, cols[perm]].T.astype(BF)
            v[slot] = V[b][:, cols].reshape(JT, NKT, HS).transpose(1, 0, 2).astype(BF)
        in_maps.append({
            "qkT": np.ascontiguousarray(qkT),
            "v": np.ascontiguousarray(v),
            "rope_c": consts["rope_c"],
            "rope_s": consts["rope_s"],
            "dmin": consts["dmin"],
            "sm": consts["sm"],
        })
    return in_maps


def _gather_outputs(per_core_outT):
    out = np.empty((B, T, C_EMB), np.float32)
    for core in range(NCORES):
        outT = per_core_outT[core]  # [PAIRS, HS, T]
        for slot in range(PAIRS):
            g = core * PAIRS + slot
            b, h = divmod(g, NH)
            out[b, :, h * HS:(h + 1) * HS] = outT[slot].T
    return out


def kernel(Q, K, V):
    from concourse.bass_utils import run_bass_kernel_spmd

    Q = np.asarray(Q, dtype=np.float32)
    K = np.asarray(K, dtype=np.float32)
    V = np.asarray(V, dtype=np.float32)

    nc = _get_program()
    in_maps = _shard_inputs(Q, K, V)
    res = run_bass_kernel_spmd(nc, in_maps, core_ids=list(range(NCORES)))
    return _gather_outputs([res.results[c]["outT"] for c in range(NCORES)])


# revision 10
# speedup vs baseline: 2.5111x; 1.1063x over previous
"""Trainium2 Bass kernel for causal self-attention with RoPE and tanh scoring.

Reference computation (per batch b, head h):
    q,k = rope(split_heads(Q)), rope(split_heads(K)); v = split_heads(V)
    scores = q @ k^T / sqrt(hs);  att = tanh(where(causal, scores, -inf))
    (masked positions become tanh(-inf) = -1 and DO contribute -1 * v)
    out = att @ v
Sharding: 32 (b,h) pairs -> 4 per core across 8 cores.

All device data is bf16; matmuls accumulate in fp32 PSUM; output fp32.
S^T formulation (scoresT[tk, tq]) per 512-wide q-chunk over the lower
triangle of k-tiles only. Causal masking is applied AFTER tanh with a
DVE min against a +-1 triangular mask on the 128-wide diagonal windows
(tanh(s) <= 1 so min(tanh, -1) = -1 exactly); fully-masked 128-col
strips are memset; fully-masked k-tiles fold into corr = -sum of the
v rows beyond the chunk (tiny step-mask matmuls).

DMA count is minimized (the shared descriptor generator costs ~630ns
per dma_start): q/k arrive in one [HS,2,T] transfer per pair plus one
host-pre-rotated copy for the RoPE pair-swap, v in one transfer, and
the output leaves in two half-pair transfers. RoPE runs in-place on
DVE per 512-wide chunk, interleaved with the previous pair's compute.
"""

import sys

if "/opt/trn_rl_repo" not in sys.path:
    sys.path.insert(0, "/opt/trn_rl_repo")

import numpy as np

B, T, C_EMB = 2, 2048, 2048
NH, HS = 16, 128
NCORES = 8
PAIRS = (B * NH) // NCORES  # 4 (b,h) pairs per core
NQ = 512                    # q-chunk width (PSUM bank = 512 fp32)
NKT = 128                   # k-tile rows
JT = T // NKT               # 16 k-tiles
NCH = T // NQ               # 4 q-chunks
SCALE = 1.0 / np.sqrt(HS)

def _host_consts():
    """Per-core constant tensors (identical on every core)."""
    import ml_dtypes
    BF = ml_dtypes.bfloat16
    i = np.arange(HS // 2, dtype=np.float64)
    freqs = 1.0 / 10000.0 ** (2.0 * i / HS)           # [64]
    t = np.arange(T, dtype=np.float64)
    ang = np.outer(freqs, t)                           # [64, T]
    cos = np.cos(ang)
    sin = np.sin(ang)
    rope_c = np.concatenate([cos, cos], axis=0).astype(BF)    # [128, T]
    rope_s = np.concatenate([-sin, sin], axis=0).astype(BF)   # [128, T]

    # diagonal-window mask: S^T[p, f] masked iff tk > tq <=> p > j within
    # the 128-wide diagonal sub-block (j = f - 128r)
    pj = np.arange(NKT)
    dmin = np.where(pj[:, None] > pj[None, :], -1.0, 1.0).astype(BF)

    # step mask for corrections: SM[p, J, c] = -1 if (128J + p) >= 512(c+1)
    sm = np.zeros((NKT, JT, NCH), np.float32)
    for j in range(JT):
        for c in range(NCH):
            tk = j * NKT + np.arange(NKT)
            sm[:, j, c] = np.where(tk >= NQ * (c + 1), -1.0, 0.0)

    return {"rope_c": rope_c, "rope_s": rope_s,
            "dmin": dmin, "sm": sm.astype(BF)}


def _build_program(reps=1):
    import concourse.bacc as bacc
    import concourse.mybir as mybir
    import concourse.tile as tile

    F32 = mybir.dt.float32
    BF16 = mybir.dt.bfloat16
    AFT = mybir.ActivationFunctionType

    nc = bacc.Bacc("TRN2", target_bir_lowering=False, debug=False)

    qk_d = nc.dram_tensor("qkT", [PAIRS, 2, HS, T], BF16, kind="ExternalInput")
    qr_d = nc.dram_tensor("qkR", [PAIRS, 2, HS, T], BF16, kind="ExternalInput")
    v_d = nc.dram_tensor("v", [PAIRS, NKT, JT, HS], BF16, kind="ExternalInput")
    rc_d = nc.dram_tensor("rope_c", [HS, T], BF16, kind="ExternalInput")
    rs_d = nc.dram_tensor("rope_s", [HS, T], BF16, kind="ExternalInput")
    dm_d = nc.dram_tensor("dmin", [NKT, NKT], BF16, kind="ExternalInput")
    sm_d = nc.dram_tensor("sm", [NKT, JT, NCH], BF16, kind="ExternalInput")
    out_d = nc.dram_tensor("outT", [PAIRS, HS, T], F32, kind="ExternalOutput")

    with tile.TileContext(nc) as tc:
        with (
            tc.tile_pool(name="consts", bufs=1) as consts,
            tc.tile_pool(name="qc", bufs=2) as q_pool,
            tc.tile_pool(name="xsp", bufs=2) as xs_pool,
            tc.tile_pool(name="vp", bufs=2) as v_pool,
            tc.tile_pool(name="att", bufs=5) as att_pool,
            tc.tile_pool(name="osb", bufs=2) as osb_pool,
            tc.tile_pool(name="corr", bufs=2) as corr_pool,
            tc.tile_pool(name="psS", bufs=3, space="PSUM") as psS,
            tc.tile_pool(name="psO", bufs=2, space="PSUM") as psO,
        ):
            rc = consts.tile([HS, T], BF16)
            rs = consts.tile([HS, T], BF16)
            dm = consts.tile([NKT, NKT], BF16)
            sm = consts.tile([NKT, JT, NCH], BF16)
            nc.scalar.dma_start(out=dm, in_=dm_d.ap())
            nc.scalar.dma_start(out=sm, in_=sm_d.ap())
            nc.scalar.dma_start(out=rc, in_=rc_d.ap())
            nc.scalar.dma_start(out=rs, in_=rs_d.ap())

            import concourse.bass as bass

            def _bcast2(ap):
                """[HS, n] slice -> [HS, 2, n] with a 0-stride middle dim."""
                return bass.AP(tensor=ap.tensor, offset=ap.offset,
                               ap=[list(ap.ap[0]), [0, 2], list(ap.ap[1])])

            def _diagwin(a, r0):
                """att [NKT, 2, NQ] -> [NKT, 2, NKT] windows at cols
                128*r0 (idx 0) and 128*(r0+1) (idx 1): mid-stride trick."""
                return bass.AP(tensor=a.tensor, offset=a.offset + NKT * r0,
                               ap=[list(a.ap[0]),
                                   [a.ap[1][0] + NKT, 2], [1, NKT]])

            def _start_load(g, chunked=False):
                """Emit the pair's three input DMAs. chunked=True splits
                q/k into 512-col pieces so the first RoPE chunk (and the
                v tile, needed by the first AV) land as early as possible
                — used for the first pair, whose loads are on the body's
                critical startup path."""
                x = q_pool.tile([HS, 2, T], BF16, tag="qk")
                xs = xs_pool.tile([HS, 2, T], BF16, tag="xs")
                vt = v_pool.tile([NKT, JT, HS], BF16, tag="v")
                src = qk_d.ap()[g].rearrange("s p t -> p s t")   # [HS, 2, T]
                srcr = qr_d.ap()[g].rearrange("s p t -> p s t")
                if chunked:
                    sl = slice(0, NQ)
                    nc.sync.dma_start(out=x[:, :, sl], in_=src[:, :, sl])
                    nc.sync.dma_start(out=xs[:, :, sl], in_=srcr[:, :, sl])
                    nc.sync.dma_start(out=vt, in_=v_d.ap()[g])
                    for ch in range(1, NCH):
                        sl = slice(ch * NQ, (ch + 1) * NQ)
                        nc.sync.dma_start(out=x[:, :, sl], in_=src[:, :, sl])
                        nc.sync.dma_start(out=xs[:, :, sl], in_=srcr[:, :, sl])
                else:
                    nc.sync.dma_start(out=x, in_=src)
                    nc.sync.dma_start(out=xs, in_=srcr)
                    nc.sync.dma_start(out=vt, in_=v_d.ap()[g])
                return [x, xs, vt]

            def _rope_chunk(st, ch):
                """In-place RoPE on a 512-col chunk: x = x*rc + rot(x)*rs."""
                x, xs, _ = st
                sl = slice(ch * NQ, (ch + 1) * NQ)
                xw = x[:, :, sl]
                xsw = xs[:, :, sl]
                nc.vector.tensor_mul(xsw, xsw, _bcast2(rs[:, sl]))
                nc.vector.tensor_mul(xw, xw, _bcast2(rc[:, sl]))
                nc.vector.tensor_add(xw, xw, xsw)

            def _one_pair(g, st, nxt):
                x, _, vt = st

                def qch(c):
                    return x[:, 0, c * NQ:(c + 1) * NQ]

                def kt(j):
                    return x[:, 1, j * NKT:(j + 1) * NKT]

                def v_of(j):
                    return vt[:, j, :]

                corr_sb = corr_pool.tile([HS, NCH], F32)
                out_sb = osb_pool.tile([HS, T], F32)

                def _emit_corr():
                    # corr[d, c] = -sum_{tk >= 512(c+1)} v[tk, d]
                    corr_ps = psO.tile([HS, NCH], mybir.dt.float32, tag="o")
                    for j in range(JT):
                        nc.tensor.matmul(corr_ps, v_of(j), sm[:, j, :],
                                         start=(j == 0), stop=(j == JT - 1))
                    nc.vector.tensor_copy(corr_sb, corr_ps)

                # ---- attention ----
                from collections import deque
                pending = deque()  # software pipeline: AV lags two groups

                def _emit_av(item):
                    o_ps, att, ja, jb, last, c = item
                    nc.tensor.matmul(o_ps, v_of(ja), att[:, 0, :],
                                     start=(ja == 0), stop=False)
                    nc.tensor.matmul(o_ps, v_of(jb), att[:, 1, :],
                                     start=False, stop=last)
                    if last:
                        # add corr and stage into the pair-level out buffer;
                        # DMA leaves in half-pair transfers
                        nc.vector.tensor_scalar_add(
                            out_sb[:, c * NQ:(c + 1) * NQ], o_ps,
                            corr_sb[:, c:c + 1])
                        if g == PAIRS - 1:
                            # last pair: per-chunk output DMAs keep the
                            # body's tail short
                            nc.sync.dma_start(
                                out=out_d.ap()[g][:, c * NQ:(c + 1) * NQ],
                                in_=out_sb[:, c * NQ:(c + 1) * NQ])
                        elif c % 2 == 1:
                            h0 = (c - 1) * NQ
                            nc.sync.dma_start(
                                out=out_d.ap()[g][:, h0:h0 + 2 * NQ],
                                in_=out_sb[:, h0:h0 + 2 * NQ])

                for c in range(NCH):
                    n_j = 4 * c + 4  # k-tiles 0..4c+3
                    o_ps = psO.tile([HS, NQ], mybir.dt.float32, tag="o")
                    last_ch = g == PAIRS - 1 and c == NCH - 1
                    for jp in range(n_j // 2):
                        ja, jb = 2 * jp, 2 * jp + 1
                        # Band tiles (j >= 4c) need masking; the pair shares a
                        # 256-aligned compute window [off, NQ).
                        r_a = ja - 4 * c
                        off = 256 if r_a >= 2 else 0
                        s = psS.tile([NKT, 2, NQ], mybir.dt.float32, tag="s")
                        for idx, j in ((0, ja), (1, jb)):
                            nc.tensor.matmul(s[:, idx, off:], kt(j),
                                             qch(c)[:, off:],
                                             start=True, stop=True)
                        att = att_pool.tile([NKT, 2, NQ], BF16)
                        if off:
                            nc.gpsimd.memset(att[:, :, 0:off], -1.0)
                        nc.scalar.activation(att[:, :, off:], s[:, :, off:],
                                             AFT.Tanh, scale=float(SCALE))
                        if r_a >= 0:
                            # post-tanh causal mask on the two 128-wide
                            # diagonal windows: min(tanh, +-1 triangle)
                            nc.vector.tensor_tensor(
                                _diagwin(att, r_a), _diagwin(att, r_a),
                                _bcast2(dm[:, :]), mybir.AluOpType.min)
                            # subtile b's fully-masked strip left of its
                            # diagonal window
                            nc.gpsimd.memset(att[:, 1, off:off + NKT], -1.0)
                        pending.append((o_ps, att, ja, jb, jb == n_j - 1, c))
                        if len(pending) > (1 if last_ch else 2):
                            _emit_av(pending.popleft())
                    if nxt is not None:
                        if c == 0:
                            _emit_corr()
                            nxt_st = _start_load(g + 1)
                            nxt.append(nxt_st)
                        # spread next pair's RoPE chunks across this pair's
                        # chunks so DVE bursts stay short
                        _rope_chunk(nxt[0], c)
                    elif c == 0:
                        _emit_corr()
                while pending:
                    _emit_av(pending.popleft())

            def _pairs_body():
                st = _start_load(0, chunked=True)
                for ch in range(NCH):
                    _rope_chunk(st, ch)
                for g in range(PAIRS):
                    nxt = [] if g + 1 < PAIRS else None
                    _one_pair(g, st, nxt)
                    st = nxt[0] if nxt else None

            if reps == 1:
                _pairs_body()
            else:
                # unroll several reps per hardware-loop iteration: the
                # For_i back edge is an all-engine barrier, so copy
                # boundaries inside the body overlap while only the outer
                # edge pays the drain/refill cost
                u = 4 if reps % 4 == 0 else (2 if reps % 2 == 0 else 1)
                with tc.For_i(0, reps // u, 1,
                              hint_engines=(mybir.EngineType.PE,
                                            mybir.EngineType.Activation,
                                            mybir.EngineType.SP,
                                            mybir.EngineType.DVE,
                                            mybir.EngineType.Pool)):
                    for _ in range(u):
                        _pairs_body()

    nc.compile()
    return nc


_PROGRAMS = {}


def _get_program(reps=1):
    if reps not in _PROGRAMS:
        _PROGRAMS[reps] = _build_program(reps)
    return _PROGRAMS[reps]


def _shard_inputs(Q, K, V):
    import ml_dtypes
    BF = ml_dtypes.bfloat16
    consts = _host_consts()
    d = np.arange(HS)
    perm = np.concatenate([d[0::2], d[1::2]])  # deinterleave head dim
    rot = np.concatenate([np.arange(64, 128), np.arange(0, 64)])

    in_maps = []
    for core in range(NCORES):
        qkT = np.empty((PAIRS, 2, HS, T), BF)
        v = np.empty((PAIRS, NKT, JT, HS), BF)
        for slot in range(PAIRS):
            g = core * PAIRS + slot
            b, h = divmod(g, NH)
            cols = h * HS + np.arange(HS)
            qkT[slot, 0] = Q[b][:, cols[perm]].T.astype(BF)
            qkT[slot, 1] = K[b][:, cols[perm]].T.astype(BF)
            v[slot] = V[b][:, cols].reshape(JT, NKT, HS).transpose(1, 0, 2).astype(BF)
        in_maps.append({
            "qkT": np.ascontiguousarray(qkT),
            "qkR": np.ascontiguousarray(qkT[:, :, rot, :]),
            "v": np.ascontiguousarray(v),
            "rope_c": consts["rope_c"],
            "rope_s": consts["rope_s"],
            "dmin": consts["dmin"],
            "sm": consts["sm"],
        })
    return in_maps


def _gather_outputs(per_core_outT):
    out = np.empty((B, T, C_EMB), np.float32)
    for core in range(NCORES):
        outT = per_core_outT[core]  # [PAIRS, HS, T]
        for slot in range(PAIRS):
            g = core * PAIRS + slot
            b, h = divmod(g, NH)
            out[b, :, h * HS:(h + 1) * HS] = outT[slot].T
    return out


def kernel(Q, K, V):
    from concourse.bass_utils import run_bass_kernel_spmd

    Q = np.asarray(Q, dtype=np.float32)
    K = np.asarray(K, dtype=np.float32)
    V = np.asarray(V, dtype=np.float32)

    nc = _get_program()
    in_maps = _shard_inputs(Q, K, V)
    res = run_bass_kernel_spmd(nc, in_maps, core_ids=list(range(NCORES)))
    return _gather_outputs([res.results[c]["outT"] for c in range(NCORES)])


# revision 14
# speedup vs baseline: 2.7663x; 1.1016x over previous
"""Trainium2 Bass kernel for causal self-attention with RoPE and tanh scoring.

Reference computation (per batch b, head h):
    q,k = rope(split_heads(Q)), rope(split_heads(K)); v = split_heads(V)
    scores = q @ k^T / sqrt(hs);  att = tanh(where(causal, scores, -inf))
    (masked positions become tanh(-inf) = -1 and DO contribute -1 * v)
    out = att @ v
Sharding: 32 (b,h) pairs -> 4 per core across 8 cores.

All device data is bf16; matmuls accumulate in fp32 PSUM; output fp32.
S^T formulation (scoresT[tk, tq]) per 512-wide q-chunk over the lower
triangle of k-tiles only. Causal masking is applied AFTER tanh with a
DVE min against a +-1 triangular mask on the 128-wide diagonal windows
(tanh(s) <= 1 so min(tanh, -1) = -1 exactly); fully-masked 128-col
strips are memset; fully-masked k-tiles fold into corr = -sum of the
v rows beyond the chunk (tiny step-mask matmuls).

DMA count is minimized (the shared descriptor generator costs ~630ns
per dma_start): q/k arrive in one [HS,2,T] transfer per pair plus one
host-pre-rotated copy for the RoPE pair-swap, v in one transfer, and
the output leaves in two half-pair transfers. RoPE runs in-place on
DVE per 512-wide chunk, interleaved with the previous pair's compute.
"""

import sys

if "/opt/trn_rl_repo" not in sys.path:
    sys.path.insert(0, "/opt/trn_rl_repo")

import numpy as np

B, T, C_EMB = 2, 2048, 2048
NH, HS = 16, 128
NCORES = 8
PAIRS = (B * NH) // NCORES  # 4 (b,h) pairs per core
NQ = 512                    # q-chunk width (PSUM bank = 512 fp32)
NKT = 128                   # k-tile rows
JT = T // NKT               # 16 k-tiles
NCH = T // NQ               # 4 q-chunks
SCALE = 1.0 / np.sqrt(HS)

def _host_consts():
    """Per-core constant tensors (identical on every core)."""
    import ml_dtypes
    BF = ml_dtypes.bfloat16
    i = np.arange(HS // 2, dtype=np.float64)
    freqs = 1.0 / 10000.0 ** (2.0 * i / HS)           # [64]
    t = np.arange(T, dtype=np.float64)
    ang = np.outer(freqs, t)                           # [64, T]
    cos = np.cos(ang)
    sin = np.sin(ang)
    rope_c = np.concatenate([cos, cos], axis=0).astype(BF)    # [128, T]
    rope_s = np.concatenate([-sin, sin], axis=0).astype(BF)   # [128, T]

    # diagonal-window mask: S^T[p, f] masked iff tk > tq <=> p > j within
    # the 128-wide diagonal sub-block (j = f - 128r)
    pj = np.arange(NKT)
    dmin = np.where(pj[:, None] > pj[None, :], -1.0, 1.0).astype(BF)

    # step mask for corrections at 128-col granularity: for the column
    # range rv (= 4c + l, covering cols 512c+128l..+127) every k-tile
    # j > rv is entirely below the causal boundary and contributes -1*v;
    # those tiles are skipped by the narrowed AV matmuls and folded here.
    sm = np.zeros((NKT, JT, JT), np.float32)
    for j in range(JT):
        for rv in range(JT):
            sm[:, j, rv] = -1.0 if j > rv else 0.0

    return {"rope_c": rope_c, "rope_s": rope_s,
            "dmin": dmin, "sm": sm.astype(BF)}


def _build_program(reps=1):
    import concourse.bacc as bacc
    import concourse.mybir as mybir
    import concourse.tile as tile

    F32 = mybir.dt.float32
    BF16 = mybir.dt.bfloat16
    AFT = mybir.ActivationFunctionType

    nc = bacc.Bacc("TRN2", target_bir_lowering=False, debug=False)

    qk_d = nc.dram_tensor("qkT", [PAIRS, 2, HS, T], BF16, kind="ExternalInput")
    qr_d = nc.dram_tensor("qkR", [PAIRS, 2, HS, T], BF16, kind="ExternalInput")
    v_d = nc.dram_tensor("v", [PAIRS, NKT, JT, HS], BF16, kind="ExternalInput")
    rc_d = nc.dram_tensor("rope_c", [HS, T], BF16, kind="ExternalInput")
    rs_d = nc.dram_tensor("rope_s", [HS, T], BF16, kind="ExternalInput")
    dm_d = nc.dram_tensor("dmin", [NKT, NKT], BF16, kind="ExternalInput")
    sm_d = nc.dram_tensor("sm", [NKT, JT, JT], BF16, kind="ExternalInput")
    out_d = nc.dram_tensor("outT", [PAIRS, HS, T], F32, kind="ExternalOutput")

    with tile.TileContext(nc) as tc:
        with (
            tc.tile_pool(name="consts", bufs=1) as consts,
            tc.tile_pool(name="qc", bufs=2) as q_pool,
            tc.tile_pool(name="xsp", bufs=2) as xs_pool,
            tc.tile_pool(name="vp", bufs=2) as v_pool,
            tc.tile_pool(name="att", bufs=5) as att_pool,
            tc.tile_pool(name="osb", bufs=2) as osb_pool,
            tc.tile_pool(name="corr", bufs=2) as corr_pool,
            tc.tile_pool(name="psS", bufs=3, space="PSUM") as psS,
            tc.tile_pool(name="psO", bufs=2, space="PSUM") as psO,
        ):
            rc = consts.tile([HS, T], BF16)
            rs = consts.tile([HS, T], BF16)
            dm = consts.tile([NKT, NKT], BF16)
            sm = consts.tile([NKT, JT, JT], BF16)
            nc.scalar.dma_start(out=dm, in_=dm_d.ap())
            nc.scalar.dma_start(out=sm, in_=sm_d.ap())
            nc.scalar.dma_start(out=rc, in_=rc_d.ap())
            nc.scalar.dma_start(out=rs, in_=rs_d.ap())

            import concourse.bass as bass

            def _bcast2(ap):
                """[HS, n] slice -> [HS, 2, n] with a 0-stride middle dim."""
                return bass.AP(tensor=ap.tensor, offset=ap.offset,
                               ap=[list(ap.ap[0]), [0, 2], list(ap.ap[1])])

            def _diagwin(a, r0):
                """att [NKT, 2, NQ] -> [NKT, 2, NKT] windows at cols
                128*r0 (idx 0) and 128*(r0+1) (idx 1): mid-stride trick."""
                return bass.AP(tensor=a.tensor, offset=a.offset + NKT * r0,
                               ap=[list(a.ap[0]),
                                   [a.ap[1][0] + NKT, 2], [1, NKT]])

            def _start_load(g, chunked=False):
                """Emit the pair's three input DMAs. chunked=True splits
                q/k into 512-col pieces so the first RoPE chunk (and the
                v tile, needed by the first AV) land as early as possible
                — used for the first pair, whose loads are on the body's
                critical startup path."""
                x = q_pool.tile([HS, 2, T], BF16, tag="qk")
                xs = xs_pool.tile([HS, 2, T], BF16, tag="xs")
                vt = v_pool.tile([NKT, JT, HS], BF16, tag="v")
                src = qk_d.ap()[g].rearrange("s p t -> p s t")   # [HS, 2, T]
                srcr = qr_d.ap()[g].rearrange("s p t -> p s t")
                if chunked:
                    sl = slice(0, NQ)
                    nc.sync.dma_start(out=x[:, :, sl], in_=src[:, :, sl])
                    nc.sync.dma_start(out=xs[:, :, sl], in_=srcr[:, :, sl])
                    nc.sync.dma_start(out=vt, in_=v_d.ap()[g])
                    for ch in range(1, NCH):
                        sl = slice(ch * NQ, (ch + 1) * NQ)
                        nc.sync.dma_start(out=x[:, :, sl], in_=src[:, :, sl])
                        nc.sync.dma_start(out=xs[:, :, sl], in_=srcr[:, :, sl])
                else:
                    nc.sync.dma_start(out=x, in_=src)
                    nc.sync.dma_start(out=xs, in_=srcr)
                    nc.sync.dma_start(out=vt, in_=v_d.ap()[g])
                return [x, xs, vt]

            def _rope_chunk(st, ch):
                """In-place RoPE on a 512-col chunk: x = x*rc + rot(x)*rs."""
                x, xs, _ = st
                sl = slice(ch * NQ, (ch + 1) * NQ)
                xw = x[:, :, sl]
                xsw = xs[:, :, sl]
                nc.vector.tensor_mul(xsw, xsw, _bcast2(rs[:, sl]))
                nc.vector.tensor_mul(xw, xw, _bcast2(rc[:, sl]))
                nc.vector.tensor_add(xw, xw, xsw)

            def _one_pair(g, st, nxt):
                x, _, vt = st

                def qch(c):
                    return x[:, 0, c * NQ:(c + 1) * NQ]

                def kt(j):
                    return x[:, 1, j * NKT:(j + 1) * NKT]

                def v_of(j):
                    return vt[:, j, :]

                corr_sb = corr_pool.tile([HS, JT], F32)
                out_sb = osb_pool.tile([HS, T], F32)

                def _emit_corr():
                    # corr[d, c] = -sum_{tk >= 512(c+1)} v[tk, d]
                    corr_ps = psO.tile([HS, JT], mybir.dt.float32, tag="o")
                    for j in range(JT):
                        nc.tensor.matmul(corr_ps, v_of(j), sm[:, j, :],
                                         start=(j == 0), stop=(j == JT - 1))
                    nc.vector.tensor_copy(corr_sb, corr_ps)

                # ---- attention ----
                from collections import deque
                pending = deque()  # software pipeline: AV lags two groups

                def _emit_av(item):
                    o_ps, att, ja, jb, last, c = item
                    # band subtiles contribute only right of their -1
                    # strip (cols >= 128r); the strip itself is folded
                    # into the 128-col-granular corr term
                    for idx, j in ((0, ja), (1, jb)):
                        lo = max(j - 4 * c, 0) * NKT
                        nc.tensor.matmul(o_ps[:, lo:], v_of(j),
                                         att[:, idx, lo:],
                                         start=(j == 0), stop=(last and idx == 1))
                    if last:
                        # add the per-128-col corr (stride-0 broadcast on
                        # the inner 128 cols) and stage into the pair-level
                        # out buffer; DMA leaves in half-pair transfers
                        osl = out_sb[:, c * NQ:(c + 1) * NQ]
                        cb = corr_sb[:, 4 * c:4 * c + 4]
                        nc.vector.tensor_tensor(
                            bass.AP(tensor=osl.tensor, offset=osl.offset,
                                    ap=[list(osl.ap[0]), [NKT, 4], [1, NKT]]),
                            bass.AP(tensor=o_ps.tensor, offset=o_ps.offset,
                                    ap=[list(o_ps.ap[0]), [NKT, 4], [1, NKT]]),
                            bass.AP(tensor=cb.tensor, offset=cb.offset,
                                    ap=[list(cb.ap[0]), [1, 4], [0, NKT]]),
                            mybir.AluOpType.add)
                        if g == PAIRS - 1:
                            # last pair: per-chunk output DMAs keep the
                            # body's tail short
                            nc.sync.dma_start(
                                out=out_d.ap()[g][:, c * NQ:(c + 1) * NQ],
                                in_=out_sb[:, c * NQ:(c + 1) * NQ])
                        elif c % 2 == 1:
                            h0 = (c - 1) * NQ
                            nc.sync.dma_start(
                                out=out_d.ap()[g][:, h0:h0 + 2 * NQ],
                                in_=out_sb[:, h0:h0 + 2 * NQ])

                for c in range(NCH):
                    n_j = 4 * c + 4  # k-tiles 0..4c+3
                    o_ps = psO.tile([HS, NQ], mybir.dt.float32, tag="o")
                    last_ch = g == PAIRS - 1 and c == NCH - 1
                    for jp in range(n_j // 2):
                        ja, jb = 2 * jp, 2 * jp + 1
                        # Band tiles (j >= 4c) need masking; the pair shares a
                        # 256-aligned compute window [off, NQ).
                        r_a = ja - 4 * c
                        off = 256 if r_a >= 2 else 0
                        s = psS.tile([NKT, 2, NQ], mybir.dt.float32, tag="s")
                        for idx, j in ((0, ja), (1, jb)):
                            nc.tensor.matmul(s[:, idx, off:], kt(j),
                                             qch(c)[:, off:],
                                             start=True, stop=True)
                        att = att_pool.tile([NKT, 2, NQ], BF16)
                        nc.scalar.activation(att[:, :, off:], s[:, :, off:],
                                             AFT.Tanh, scale=float(SCALE))
                        if r_a >= 0:
                            # post-tanh causal mask on the two 128-wide
                            # diagonal windows: min(tanh, +-1 triangle);
                            # everything left of a window is skipped by
                            # the narrowed AV and folded into corr
                            nc.vector.tensor_tensor(
                                _diagwin(att, r_a), _diagwin(att, r_a),
                                _bcast2(dm[:, :]), mybir.AluOpType.min)
                        pending.append((o_ps, att, ja, jb, jb == n_j - 1, c))
                        if len(pending) > (1 if last_ch else 2):
                            _emit_av(pending.popleft())
                    if nxt is not None:
                        if c == 0:
                            _emit_corr()
                            nxt_st = _start_load(g + 1)
                            nxt.append(nxt_st)
                        # spread next pair's RoPE chunks across this pair's
                        # chunks so DVE bursts stay short
                        _rope_chunk(nxt[0], c)
                    elif c == 0:
                        _emit_corr()
                while pending:
                    _emit_av(pending.popleft())

            def _pairs_body():
                st = _start_load(0, chunked=True)
                for ch in range(NCH):
                    _rope_chunk(st, ch)
                for g in range(PAIRS):
                    nxt = [] if g + 1 < PAIRS else None
                    _one_pair(g, st, nxt)
                    st = nxt[0] if nxt else None

            if reps == 1:
                _pairs_body()
            else:
                # unroll several reps per hardware-loop iteration: the
                # For_i back edge is an all-engine barrier, so copy
                # boundaries inside the body overlap while only the outer
                # edge pays the drain/refill cost
                u = 4 if reps % 4 == 0 else (2 if reps % 2 == 0 else 1)
                with tc.For_i(0, reps // u, 1,
                              hint_engines=(mybir.EngineType.PE,
                                            mybir.EngineType.Activation,
                                            mybir.EngineType.SP,
                                            mybir.EngineType.DVE,
                                            mybir.EngineType.Pool)):
                    for _ in range(u):
                        _pairs_body()

    nc.compile()
    return nc


_PROGRAMS = {}


def _get_program(reps=1):
    if reps not in _PROGRAMS:
        _PROGRAMS[reps] = _build_program(reps)
    return _PROGRAMS[reps]


def _shard_inputs(Q, K, V):
    import ml_dtypes
    BF = ml_dtypes.bfloat16
    consts = _host_consts()
    d = np.arange(HS)
    perm = np.concatenate([d[0::2], d[1::2]])  # deinterleave head dim
    rot = np.concatenate([np.arange(64, 128), np.arange(0, 64)])

    in_maps = []
    for core in range(NCORES):
        qkT = np.empty((PAIRS, 2, HS, T), BF)
        v = np.empty((PAIRS, NKT, JT, HS), BF)
        for slot in range(PAIRS):
            g = core * PAIRS + slot
            b, h = divmod(g, NH)
            cols = h * HS + np.arange(HS)
            qkT[slot, 0] = Q[b][:, cols[perm]].T.astype(BF)
            qkT[slot, 1] = K[b][:, cols[perm]].T.astype(BF)
            v[slot] = V[b][:, cols].reshape(JT, NKT, HS).transpose(1, 0, 2).astype(BF)
        in_maps.append({
            "qkT": np.ascontiguousarray(qkT),
            "qkR": np.ascontiguousarray(qkT[:, :, rot, :]),
            "v": np.ascontiguousarray(v),
            "rope_c": consts["rope_c"],
            "rope_s": consts["rope_s"],
            "dmin": consts["dmin"],
            "sm": consts["sm"],
        })
    return in_maps


def _gather_outputs(per_core_outT):
    out = np.empty((B, T, C_EMB), np.float32)
    for core in range(NCORES):
        outT = per_core_outT[core]  # [PAIRS, HS, T]
        for slot in range(PAIRS):
            g = core * PAIRS + slot
            b, h = divmod(g, NH)
            out[b, :, h * HS:(h + 1) * HS] = outT[slot].T
    return out


def kernel(Q, K, V):
    from concourse.bass_utils import run_bass_kernel_spmd

    Q = np.asarray(Q, dtype=np.float32)
    K = np.asarray(K, dtype=np.float32)
    V = np.asarray(V, dtype=np.float32)

    nc = _get_program()
    in_maps = _shard_inputs(Q, K, V)
    res = run_bass_kernel_spmd(nc, in_maps, core_ids=list(range(NCORES)))
    return _gather_outputs([res.results[c]["outT"] for c in range(NCORES)])


# revision 18
# speedup vs baseline: 2.9459x; 1.0649x over previous
"""Trainium2 Bass kernel for causal self-attention with RoPE and tanh scoring.

Reference computation (per batch b, head h):
    q,k = rope(split_heads(Q)), rope(split_heads(K)); v = split_heads(V)
    scores = q @ k^T / sqrt(hs);  att = tanh(where(causal, scores, -inf))
    (masked positions become tanh(-inf) = -1 and DO contribute -1 * v)
    out = att @ v
Sharding: 32 (b,h) pairs -> 4 per core across 8 cores.

All device data is bf16; matmuls accumulate in fp32 PSUM; output fp32.
S^T formulation (scoresT[tk, tq]) per 512-wide q-chunk over the lower
triangle of k-tiles only. Causal masking is applied AFTER tanh with a
DVE min against a +-1 triangular mask on the 128-wide diagonal windows
(tanh(s) <= 1 so min(tanh, -1) = -1 exactly); fully-masked 128-col
strips are memset; fully-masked k-tiles fold into corr = -sum of the
v rows beyond the chunk (tiny step-mask matmuls).

DMA count is minimized (the shared descriptor generator costs ~630ns
per dma_start): q/k arrive in one [HS,2,T] transfer per pair plus one
host-pre-rotated copy for the RoPE pair-swap, v in one transfer, and
the output leaves in two half-pair transfers. RoPE runs in-place on
DVE per 512-wide chunk, interleaved with the previous pair's compute.
"""

import sys

if "/opt/trn_rl_repo" not in sys.path:
    sys.path.insert(0, "/opt/trn_rl_repo")

import numpy as np

B, T, C_EMB = 2, 2048, 2048
NH, HS = 16, 128
NCORES = 8
PAIRS = (B * NH) // NCORES  # 4 (b,h) pairs per core
NQ = 512                    # q-chunk width (PSUM bank = 512 fp32)
NKT = 128                   # k-tile rows
JT = T // NKT               # 16 k-tiles
NCH = T // NQ               # 4 q-chunks
SCALE = 1.0 / np.sqrt(HS)

def _host_consts():
    """Per-core constant tensors (identical on every core)."""
    import ml_dtypes
    BF = ml_dtypes.bfloat16
    i = np.arange(HS // 2, dtype=np.float64)
    freqs = 1.0 / 10000.0 ** (2.0 * i / HS)           # [64]
    t = np.arange(T, dtype=np.float64)
    ang = np.outer(freqs, t)                           # [64, T]
    cos = np.cos(ang)
    sin = np.sin(ang)
    rope_c = np.concatenate([cos, cos], axis=0).astype(BF)    # [128, T]
    rope_s = np.concatenate([-sin, sin], axis=0).astype(BF)   # [128, T]

    # diagonal-window mask: S^T[p, f] masked iff tk > tq <=> p > j within
    # the 128-wide diagonal sub-block (j = f - 128r)
    pj = np.arange(NKT)
    dmin = np.where(pj[:, None] > pj[None, :], -1.0, 1.0).astype(BF)

    # step mask for corrections at 128-col granularity: for the column
    # range rv (= 4c + l, covering cols 512c+128l..+127) every k-tile
    # j > rv is entirely below the causal boundary and contributes -1*v;
    # those tiles are skipped by the narrowed AV matmuls and folded here.
    sm = np.zeros((NKT, JT, JT), np.float32)
    for j in range(JT):
        for rv in range(JT):
            sm[:, j, rv] = -1.0 if j > rv else 0.0

    return {"rope_c": rope_c, "rope_s": rope_s,
            "dmin": dmin, "sm": sm.astype(BF)}


def _build_program(reps=1):
    import concourse.bacc as bacc
    import concourse.mybir as mybir
    import concourse.tile as tile

    F32 = mybir.dt.float32
    BF16 = mybir.dt.bfloat16
    AFT = mybir.ActivationFunctionType

    nc = bacc.Bacc("TRN2", target_bir_lowering=False, debug=False)

    qk_d = nc.dram_tensor("qkT", [PAIRS, 2, HS, T], BF16, kind="ExternalInput")
    qr_d = nc.dram_tensor("qkR", [PAIRS, 2, HS, T], BF16, kind="ExternalInput")
    v_d = nc.dram_tensor("v", [PAIRS, NKT, JT, HS], BF16, kind="ExternalInput")
    rc_d = nc.dram_tensor("rope_c", [HS, T], BF16, kind="ExternalInput")
    rs_d = nc.dram_tensor("rope_s", [HS, T], BF16, kind="ExternalInput")
    dm_d = nc.dram_tensor("dmin", [NKT, NKT], BF16, kind="ExternalInput")
    sm_d = nc.dram_tensor("sm", [NKT, JT, JT], BF16, kind="ExternalInput")
    out_d = nc.dram_tensor("outT", [PAIRS, HS, T], F32, kind="ExternalOutput")

    with tile.TileContext(nc) as tc:
        with (
            tc.tile_pool(name="consts", bufs=1) as consts,
            tc.tile_pool(name="qc", bufs=2) as q_pool,
            tc.tile_pool(name="xsp", bufs=2) as xs_pool,
            tc.tile_pool(name="vp", bufs=2) as v_pool,
            tc.tile_pool(name="att", bufs=5) as att_pool,
            tc.tile_pool(name="osb", bufs=2) as osb_pool,
            tc.tile_pool(name="corr", bufs=2) as corr_pool,
            tc.tile_pool(name="psS", bufs=3, space="PSUM") as psS,
            tc.tile_pool(name="psO", bufs=2, space="PSUM") as psO,
        ):
            rc = consts.tile([HS, T], BF16)
            rs = consts.tile([HS, T], BF16)
            dm = consts.tile([NKT, NKT], BF16)
            sm = consts.tile([NKT, JT, JT], BF16)
            nc.scalar.dma_start(out=dm, in_=dm_d.ap())
            nc.scalar.dma_start(out=sm, in_=sm_d.ap())
            nc.scalar.dma_start(out=rc, in_=rc_d.ap())
            nc.scalar.dma_start(out=rs, in_=rs_d.ap())



            import concourse.bass as bass

            def _bcast2(ap):
                """[HS, n] slice -> [HS, 2, n] with a 0-stride middle dim."""
                return bass.AP(tensor=ap.tensor, offset=ap.offset,
                               ap=[list(ap.ap[0]), [0, 2], list(ap.ap[1])])

            def _diagwin(a, r0):
                """att [NKT, 2, NQ] -> [NKT, 2, NKT] windows at cols
                128*r0 (idx 0) and 128*(r0+1) (idx 1): mid-stride trick."""
                return bass.AP(tensor=a.tensor, offset=a.offset + NKT * r0,
                               ap=[list(a.ap[0]),
                                   [a.ap[1][0] + NKT, 2], [1, NKT]])

            def _start_load(g, chunked=False):
                """Emit the pair's three input DMAs. chunked=True splits
                q/k into 512-col pieces so the first RoPE chunk (and the
                v tile, needed by the first AV) land as early as possible
                — used for the first pair, whose loads are on the body's
                critical startup path."""
                x = q_pool.tile([HS, 2, T], BF16, tag="qk")
                xs = xs_pool.tile([HS, 2, T], BF16, tag="xs")
                vt = v_pool.tile([NKT, JT, HS], BF16, tag="v")
                src = qk_d.ap()[g].rearrange("s p t -> p s t")   # [HS, 2, T]
                srcr = qr_d.ap()[g].rearrange("s p t -> p s t")
                if chunked:
                    sl = slice(0, NQ)
                    nc.sync.dma_start(out=x[:, :, sl], in_=src[:, :, sl])
                    nc.sync.dma_start(out=xs[:, :, sl], in_=srcr[:, :, sl])
                    nc.sync.dma_start(out=vt, in_=v_d.ap()[g])
                    for ch in range(1, NCH):
                        sl = slice(ch * NQ, (ch + 1) * NQ)
                        nc.sync.dma_start(out=x[:, :, sl], in_=src[:, :, sl])
                        nc.sync.dma_start(out=xs[:, :, sl], in_=srcr[:, :, sl])
                else:
                    nc.sync.dma_start(out=x, in_=src)
                    nc.sync.dma_start(out=xs, in_=srcr)
                    nc.sync.dma_start(out=vt, in_=v_d.ap()[g])
                return [x, xs, vt]

            def _rope_chunk(st, ch):
                """In-place RoPE on a 512-col chunk: x = x*rc + rot(x)*rs."""
                x, xs, _ = st
                sl = slice(ch * NQ, (ch + 1) * NQ)
                xw = x[:, :, sl]
                xsw = xs[:, :, sl]
                nc.vector.tensor_mul(xsw, xsw, _bcast2(rs[:, sl]))
                nc.vector.tensor_mul(xw, xw, _bcast2(rc[:, sl]))
                nc.vector.tensor_add(xw, xw, xsw)

            def _one_pair(g, st, nxt):
                x, _, vt = st

                def qch(c):
                    return x[:, 0, c * NQ:(c + 1) * NQ]

                def kt(j):
                    return x[:, 1, j * NKT:(j + 1) * NKT]

                def v_of(j):
                    return vt[:, j, :]

                corr_sb = corr_pool.tile([HS, JT], F32)
                out_sb = osb_pool.tile([HS, T], F32)

                def _emit_corr():
                    # corr[d, c] = -sum_{tk >= 512(c+1)} v[tk, d]
                    corr_ps = psO.tile([HS, JT], mybir.dt.float32, tag="o")
                    for j in range(JT):
                        nc.tensor.matmul(corr_ps, v_of(j), sm[:, j, :],
                                         start=(j == 0), stop=(j == JT - 1))
                    nc.vector.tensor_copy(corr_sb, corr_ps)

                # ---- attention ----
                from collections import deque
                pending = deque()  # software pipeline: AV lags two groups

                def _emit_av(item):
                    o_ps, att, ja, jb, last, c = item
                    # band subtiles contribute only right of their -1
                    # strip (cols >= 128r); the strip itself is folded
                    # into the 128-col-granular corr term
                    for idx, j in ((0, ja), (1, jb)):
                        lo = max(j - 4 * c, 0) * NKT
                        nc.tensor.matmul(o_ps[:, lo:], v_of(j),
                                         att[:, idx, lo:],
                                         start=(j == 0), stop=(last and idx == 1))
                    if last:
                        # add the per-128-col corr (stride-0 broadcast on
                        # the inner 128 cols) and stage into the pair-level
                        # out buffer; DMA leaves in half-pair transfers
                        osl = out_sb[:, c * NQ:(c + 1) * NQ]
                        cb = corr_sb[:, 4 * c:4 * c + 4]
                        nc.vector.tensor_tensor(
                            bass.AP(tensor=osl.tensor, offset=osl.offset,
                                    ap=[list(osl.ap[0]), [NKT, 4], [1, NKT]]),
                            bass.AP(tensor=o_ps.tensor, offset=o_ps.offset,
                                    ap=[list(o_ps.ap[0]), [NKT, 4], [1, NKT]]),
                            bass.AP(tensor=cb.tensor, offset=cb.offset,
                                    ap=[list(cb.ap[0]), [1, 4], [0, NKT]]),
                            mybir.AluOpType.add)
                        if g == PAIRS - 1:
                            # last pair: per-chunk output DMAs keep the
                            # body's tail short
                            nc.sync.dma_start(
                                out=out_d.ap()[g][:, c * NQ:(c + 1) * NQ],
                                in_=out_sb[:, c * NQ:(c + 1) * NQ])
                        elif c % 2 == 1:
                            h0 = (c - 1) * NQ
                            nc.sync.dma_start(
                                out=out_d.ap()[g][:, h0:h0 + 2 * NQ],
                                in_=out_sb[:, h0:h0 + 2 * NQ])

                for c in range(NCH):
                    n_j = 4 * c + 4  # k-tiles 0..4c+3
                    o_ps = psO.tile([HS, NQ], mybir.dt.float32, tag="o")
                    last_ch = g == PAIRS - 1 and c == NCH - 1
                    for jp in range(n_j // 2):
                        ja, jb = 2 * jp, 2 * jp + 1
                        # Band subtiles (j >= 4c) only need scores right of
                        # their fully-masked strip (cols >= 128r).
                        r_a = ja - 4 * c
                        lo_a, lo_b = max(r_a, 0) * NKT, max(r_a + 1, 0) * NKT if r_a >= 0 else 0
                        s = psS.tile([NKT, 2, NQ], mybir.dt.float32, tag="s")
                        for idx, lo in ((0, lo_a), (1, lo_b)):
                            nc.tensor.matmul(s[:, idx, lo:], kt((ja, jb)[idx]),
                                             qch(c)[:, lo:],
                                             start=True, stop=True)
                        att = att_pool.tile([NKT, 2, NQ], BF16)
                        # tanh windows match exactly what the matmuls wrote:
                        # a uniform window from the wider strip plus a small
                        # strip-activation for subtile a's extra 128 cols
                        nc.scalar.activation(att[:, :, lo_b:], s[:, :, lo_b:],
                                             AFT.Tanh, scale=float(SCALE))
                        if lo_b > lo_a:
                            nc.scalar.activation(att[:, 0, lo_a:lo_b],
                                                 s[:, 0, lo_a:lo_b],
                                                 AFT.Tanh, scale=float(SCALE))
                        if r_a >= 0:
                            # post-tanh causal mask on the two 128-wide
                            # diagonal windows: min(tanh, +-1 triangle);
                            # everything left of a window is skipped by
                            # the narrowed AV and folded into corr
                            nc.vector.tensor_tensor(
                                _diagwin(att, r_a), _diagwin(att, r_a),
                                _bcast2(dm[:, :]), mybir.AluOpType.min)
                        pending.append((o_ps, att, ja, jb, jb == n_j - 1, c))
                        if len(pending) > (1 if last_ch else 2):
                            _emit_av(pending.popleft())
                    if nxt is not None:
                        if c == 0:
                            _emit_corr()
                            nxt_st = _start_load(g + 1)
                            nxt.append(nxt_st)
                        # spread next pair's RoPE chunks across this pair's
                        # chunks so DVE bursts stay short
                        _rope_chunk(nxt[0], c)
                    elif c == 0:
                        _emit_corr()
                while pending:
                    _emit_av(pending.popleft())

            def _pairs_body():
                st = _start_load(0, chunked=True)
                for ch in range(NCH):
                    _rope_chunk(st, ch)
                for g in range(PAIRS):
                    nxt = [] if g + 1 < PAIRS else None
                    _one_pair(g, st, nxt)
                    st = nxt[0] if nxt else None

            if reps == 1:
                _pairs_body()
            else:
                # unroll several reps per hardware-loop iteration: the
                # For_i back edge is an all-engine barrier, so copy
                # boundaries inside the body overlap while only the outer
                # edge pays the drain/refill cost
                u = 4 if reps % 4 == 0 else (2 if reps % 2 == 0 else 1)
                with tc.For_i(0, reps // u, 1,
                              hint_engines=(mybir.EngineType.PE,
                                            mybir.EngineType.Activation,
                                            mybir.EngineType.SP,
                                            mybir.EngineType.DVE,
                                            mybir.EngineType.Pool)):
                    for _ in range(u):
                        _pairs_body()

    nc.compile()
    return nc


_PROGRAMS = {}


def _get_program(reps=1):
    if reps not in _PROGRAMS:
        _PROGRAMS[reps] = _build_program(reps)
    return _PROGRAMS[reps]


def _shard_inputs(Q, K, V):
    import ml_dtypes
    BF = ml_dtypes.bfloat16
    consts = _host_consts()
    d = np.arange(HS)
    perm = np.concatenate([d[0::2], d[1::2]])  # deinterleave head dim
    rot = np.concatenate([np.arange(64, 128), np.arange(0, 64)])

    in_maps = []
    for core in range(NCORES):
        qkT = np.empty((PAIRS, 2, HS, T), BF)
        v = np.empty((PAIRS, NKT, JT, HS), BF)
        for slot in range(PAIRS):
            g = core * PAIRS + slot
            b, h = divmod(g, NH)
            cols = h * HS + np.arange(HS)
            qkT[slot, 0] = Q[b][:, cols[perm]].T.astype(BF)
            qkT[slot, 1] = K[b][:, cols[perm]].T.astype(BF)
            v[slot] = V[b][:, cols].reshape(JT, NKT, HS).transpose(1, 0, 2).astype(BF)
        in_maps.append({
            "qkT": np.ascontiguousarray(qkT),
            "qkR": np.ascontiguousarray(qkT[:, :, rot, :]),
            "v": np.ascontiguousarray(v),
            "rope_c": consts["rope_c"],
            "rope_s": consts["rope_s"],
            "dmin": consts["dmin"],
            "sm": consts["sm"],
        })
    return in_maps


def _gather_outputs(per_core_outT):
    out = np.empty((B, T, C_EMB), np.float32)
    for core in range(NCORES):
        outT = per_core_outT[core]  # [PAIRS, HS, T]
        for slot in range(PAIRS):
            g = core * PAIRS + slot
            b, h = divmod(g, NH)
            out[b, :, h * HS:(h + 1) * HS] = outT[slot].T
    return out


def kernel(Q, K, V):
    from concourse.bass_utils import run_bass_kernel_spmd

    Q = np.asarray(Q, dtype=np.float32)
    K = np.asarray(K, dtype=np.float32)
    V = np.asarray(V, dtype=np.float32)

    nc = _get_program()
    in_maps = _shard_inputs(Q, K, V)
    res = run_bass_kernel_spmd(nc, in_maps, core_ids=list(range(NCORES)))
    return _gather_outputs([res.results[c]["outT"] for c in range(NCORES)])


# revision 22
# speedup vs baseline: 3.0945x; 1.0504x over previous
"""Trainium2 Bass kernel for causal self-attention with RoPE and tanh scoring.

Reference computation (per batch b, head h):
    q,k = rope(split_heads(Q)), rope(split_heads(K)); v = split_heads(V)
    scores = q @ k^T / sqrt(hs);  att = tanh(where(causal, scores, -inf))
    (masked positions become tanh(-inf) = -1 and DO contribute -1 * v)
    out = att @ v
Sharding: 32 (b,h) pairs -> 4 per core across 8 cores.

All device data is bf16; matmuls accumulate in fp32 PSUM; output fp32.
S^T formulation (scoresT[tk, tq]) per 512-wide q-chunk over the lower
triangle of k-tiles only. Causal masking is applied AFTER tanh with a
DVE min against a +-1 triangular mask on the 128-wide diagonal windows
(tanh(s) <= 1 so min(tanh, -1) = -1 exactly); fully-masked 128-col
strips are memset; fully-masked k-tiles fold into corr = -sum of the
v rows beyond the chunk (tiny step-mask matmuls).

DMA count is minimized (the shared descriptor generator costs ~630ns
per dma_start): q/k arrive in one [HS,2,T] transfer per pair plus one
host-pre-rotated copy for the RoPE pair-swap, v in one transfer, and
the output leaves in two half-pair transfers. RoPE runs in-place on
DVE per 512-wide chunk, interleaved with the previous pair's compute.
"""

import sys

if "/opt/trn_rl_repo" not in sys.path:
    sys.path.insert(0, "/opt/trn_rl_repo")

import numpy as np

B, T, C_EMB = 2, 2048, 2048
NH, HS = 16, 128
NCORES = 8
PAIRS = (B * NH) // NCORES  # 4 (b,h) pairs per core
NQ = 512                    # q-chunk width (PSUM bank = 512 fp32)
NKT = 128                   # k-tile rows
JT = T // NKT               # 16 k-tiles
NCH = T // NQ               # 4 q-chunks
SCALE = 1.0 / np.sqrt(HS)

def _host_consts():
    """Per-core constant tensors (identical on every core)."""
    import ml_dtypes
    BF = ml_dtypes.bfloat16
    i = np.arange(HS // 2, dtype=np.float64)
    freqs = 1.0 / 10000.0 ** (2.0 * i / HS)           # [64]
    t = np.arange(T, dtype=np.float64)
    ang = np.outer(freqs, t)                           # [64, T]
    cos = np.cos(ang)
    sin = np.sin(ang)
    rope_c = np.concatenate([cos, cos], axis=0).astype(BF)    # [128, T]
    rope_s = np.concatenate([-sin, sin], axis=0).astype(BF)   # [128, T]

    # diagonal-window mask: S^T[p, f] masked iff tk > tq <=> p > j within
    # the 128-wide diagonal sub-block (j = f - 128r)
    pj = np.arange(NKT)
    dmin = np.where(pj[:, None] > pj[None, :], -1.0, 1.0).astype(BF)

    return {"rope_c": rope_c, "rope_s": rope_s, "dmin": dmin}


def _build_program(reps=1):
    import concourse.bacc as bacc
    import concourse.mybir as mybir
    import concourse.tile as tile

    F32 = mybir.dt.float32
    BF16 = mybir.dt.bfloat16
    AFT = mybir.ActivationFunctionType

    nc = bacc.Bacc("TRN2", target_bir_lowering=False, debug=False)

    qk_d = nc.dram_tensor("qkT", [PAIRS, 2, HS, T], BF16, kind="ExternalInput")
    qr_d = nc.dram_tensor("qkR", [PAIRS, 2, HS, T], BF16, kind="ExternalInput")
    v_d = nc.dram_tensor("v", [PAIRS, NKT, JT, HS], BF16, kind="ExternalInput")
    rc_d = nc.dram_tensor("rope_c", [HS, T], BF16, kind="ExternalInput")
    rs_d = nc.dram_tensor("rope_s", [HS, T], BF16, kind="ExternalInput")
    dm_d = nc.dram_tensor("dmin", [NKT, NKT], BF16, kind="ExternalInput")
    vt_d = nc.dram_tensor("vT", [PAIRS, HS, T], BF16, kind="ExternalInput")
    out_d = nc.dram_tensor("outT", [PAIRS, HS, T], F32, kind="ExternalOutput")

    with tile.TileContext(nc) as tc:
        with (
            tc.tile_pool(name="consts", bufs=1) as consts,
            tc.tile_pool(name="qc", bufs=2) as q_pool,
            tc.tile_pool(name="xsp", bufs=2) as xs_pool,
            tc.tile_pool(name="vp", bufs=2) as v_pool,
            tc.tile_pool(name="vtp", bufs=2) as vt_pool,
            tc.tile_pool(name="att", bufs=5) as att_pool,
            tc.tile_pool(name="osb", bufs=2) as osb_pool,
            tc.tile_pool(name="corr", bufs=2) as corr_pool,
            tc.tile_pool(name="psS", bufs=3, space="PSUM") as psS,
            tc.tile_pool(name="psO", bufs=2, space="PSUM") as psO,
        ):
            rc = consts.tile([HS, T], BF16)
            rs = consts.tile([HS, T], BF16)
            dm = consts.tile([NKT, NKT], BF16)
            nc.scalar.dma_start(out=dm, in_=dm_d.ap())
            nc.scalar.dma_start(out=rc, in_=rc_d.ap())
            nc.scalar.dma_start(out=rs, in_=rs_d.ap())



            import concourse.bass as bass

            def _bcast2(ap):
                """[HS, n] slice -> [HS, 2, n] with a 0-stride middle dim."""
                return bass.AP(tensor=ap.tensor, offset=ap.offset,
                               ap=[list(ap.ap[0]), [0, 2], list(ap.ap[1])])

            def _diagwin(a, r0):
                """att [NKT, 2, NQ] -> [NKT, 2, NKT] windows at cols
                128*r0 (idx 0) and 128*(r0+1) (idx 1): mid-stride trick."""
                return bass.AP(tensor=a.tensor, offset=a.offset + NKT * r0,
                               ap=[list(a.ap[0]),
                                   [a.ap[1][0] + NKT, 2], [1, NKT]])

            def _start_load(g, chunked=False):
                """Emit the pair's three input DMAs. chunked=True splits
                q/k into 512-col pieces so the first RoPE chunk (and the
                v tile, needed by the first AV) land as early as possible
                — used for the first pair, whose loads are on the body's
                critical startup path."""
                x = q_pool.tile([HS, 2, T], BF16, tag="qk")
                xs = xs_pool.tile([HS, 2, T], BF16, tag="xs")
                vt = v_pool.tile([NKT, JT, HS], BF16, tag="v")
                vtt = vt_pool.tile([HS, T], BF16, tag="vT")
                src = qk_d.ap()[g].rearrange("s p t -> p s t")   # [HS, 2, T]
                srcr = qr_d.ap()[g].rearrange("s p t -> p s t")
                if chunked:
                    sl = slice(0, NQ)
                    nc.sync.dma_start(out=x[:, :, sl], in_=src[:, :, sl])
                    nc.sync.dma_start(out=xs[:, :, sl], in_=srcr[:, :, sl])
                    nc.sync.dma_start(out=vt, in_=v_d.ap()[g])
                    nc.sync.dma_start(out=vtt, in_=vt_d.ap()[g])
                    for ch in range(1, NCH):
                        sl = slice(ch * NQ, (ch + 1) * NQ)
                        nc.sync.dma_start(out=x[:, :, sl], in_=src[:, :, sl])
                        nc.sync.dma_start(out=xs[:, :, sl], in_=srcr[:, :, sl])
                else:
                    nc.sync.dma_start(out=x, in_=src)
                    nc.sync.dma_start(out=xs, in_=srcr)
                    nc.sync.dma_start(out=vt, in_=v_d.ap()[g])
                    nc.sync.dma_start(out=vtt, in_=vt_d.ap()[g])
                return [x, xs, vt, vtt]

            def _rope_chunk(st, ch):
                """In-place RoPE on a 512-col chunk: x = x*rc + rot(x)*rs."""
                x, xs = st[0], st[1]
                sl = slice(ch * NQ, (ch + 1) * NQ)
                xw = x[:, :, sl]
                xsw = xs[:, :, sl]
                nc.vector.tensor_mul(xsw, xsw, _bcast2(rs[:, sl]))
                nc.vector.tensor_mul(xw, xw, _bcast2(rc[:, sl]))
                nc.vector.tensor_add(xw, xw, xsw)

            def _one_pair(g, st, nxt):
                x, _, vt, vtt = st

                def qch(c):
                    return x[:, 0, c * NQ:(c + 1) * NQ]

                def kt(j):
                    return x[:, 1, j * NKT:(j + 1) * NKT]

                def v_of(j):
                    return vt[:, j, :]

                wa = corr_pool.tile([HS, 2, 2 * JT], F32)
                corr_sb = wa[:, 0, :]  # scan result lands in lane 0
                out_sb = osb_pool.tile([HS, T], F32)

                def _emit_corr():
                    # negated 128-col block sums of v (one DVE reduce over
                    # the transposed copy), then an inclusive suffix scan
                    # (log-step shifted adds over a zero pad); use sites
                    # read shifted by one column for the exclusive sum
                    # corr[rv] = -sum of v rows in tiles j > rv
                    nc.vector.memset(wa, 0.0)
                    vv = vtt[:, :]
                    nc.vector.tensor_reduce(
                        wa[:, 0, 0:JT],
                        bass.AP(tensor=vv.tensor, offset=vv.offset,
                                ap=[list(vv.ap[0]), [NKT, JT], [1, NKT]]),
                        mybir.AxisListType.X, mybir.AluOpType.add,
                        negate=True)
                    pp = [wa[:, 0, :], wa[:, 1, :]]
                    for i, sh in enumerate((1, 2, 4, 8)):
                        a, b = pp[i % 2], pp[(i + 1) % 2]
                        nc.vector.tensor_add(b[:, 0:JT], a[:, 0:JT],
                                             a[:, sh:JT + sh])


                # ---- attention ----
                from collections import deque
                pending = deque()  # software pipeline: AV lags two groups

                def _emit_av(item):
                    o_ps, att, ja, jb, last, c = item
                    # band subtiles contribute only right of their -1
                    # strip (cols >= 128r); the strip itself is folded
                    # into the 128-col-granular corr term
                    for idx, j in ((0, ja), (1, jb)):
                        lo = max(j - 4 * c, 0) * NKT
                        nc.tensor.matmul(o_ps[:, lo:], v_of(j),
                                         att[:, idx, lo:],
                                         start=(j == 0), stop=(last and idx == 1))
                    if last:
                        # add the per-128-col corr (stride-0 broadcast on
                        # the inner 128 cols) and stage into the pair-level
                        # out buffer; DMA leaves in half-pair transfers
                        osl = out_sb[:, c * NQ:(c + 1) * NQ]
                        cb = corr_sb[:, 4 * c + 1:4 * c + 5]
                        nc.vector.tensor_tensor(
                            bass.AP(tensor=osl.tensor, offset=osl.offset,
                                    ap=[list(osl.ap[0]), [NKT, 4], [1, NKT]]),
                            bass.AP(tensor=o_ps.tensor, offset=o_ps.offset,
                                    ap=[list(o_ps.ap[0]), [NKT, 4], [1, NKT]]),
                            bass.AP(tensor=cb.tensor, offset=cb.offset,
                                    ap=[list(cb.ap[0]), [1, 4], [0, NKT]]),
                            mybir.AluOpType.add)
                        if g == PAIRS - 1:
                            # last pair: per-chunk output DMAs keep the
                            # body's tail short
                            nc.sync.dma_start(
                                out=out_d.ap()[g][:, c * NQ:(c + 1) * NQ],
                                in_=out_sb[:, c * NQ:(c + 1) * NQ])
                        elif c % 2 == 1:
                            h0 = (c - 1) * NQ
                            nc.sync.dma_start(
                                out=out_d.ap()[g][:, h0:h0 + 2 * NQ],
                                in_=out_sb[:, h0:h0 + 2 * NQ])

                for c in range(NCH):
                    n_j = 4 * c + 4  # k-tiles 0..4c+3
                    o_ps = psO.tile([HS, NQ], mybir.dt.float32, tag="o")
                    last_ch = g == PAIRS - 1 and c == NCH - 1
                    for jp in range(n_j // 2):
                        ja, jb = 2 * jp, 2 * jp + 1
                        # Band subtiles (j >= 4c) only need scores right of
                        # their fully-masked strip (cols >= 128r).
                        r_a = ja - 4 * c
                        lo_a, lo_b = max(r_a, 0) * NKT, max(r_a + 1, 0) * NKT if r_a >= 0 else 0
                        s = psS.tile([NKT, 2, NQ], mybir.dt.float32, tag="s")
                        for idx, lo in ((0, lo_a), (1, lo_b)):
                            nc.tensor.matmul(s[:, idx, lo:], kt((ja, jb)[idx]),
                                             qch(c)[:, lo:],
                                             start=True, stop=True)
                        att = att_pool.tile([NKT, 2, NQ], BF16)
                        # tanh windows match exactly what the matmuls wrote:
                        # a uniform window from the wider strip plus a small
                        # strip-activation for subtile a's extra 128 cols
                        nc.scalar.activation(att[:, :, lo_b:], s[:, :, lo_b:],
                                             AFT.Tanh, scale=float(SCALE))
                        if lo_b > lo_a:
                            nc.scalar.activation(att[:, 0, lo_a:lo_b],
                                                 s[:, 0, lo_a:lo_b],
                                                 AFT.Tanh, scale=float(SCALE))
                        if r_a >= 0:
                            # post-tanh causal mask on the two 128-wide
                            # diagonal windows: min(tanh, +-1 triangle);
                            # everything left of a window is skipped by
                            # the narrowed AV and folded into corr
                            nc.vector.tensor_tensor(
                                _diagwin(att, r_a), _diagwin(att, r_a),
                                _bcast2(dm[:, :]), mybir.AluOpType.min)
                        pending.append((o_ps, att, ja, jb, jb == n_j - 1, c))
                        if len(pending) > (1 if last_ch else 2):
                            _emit_av(pending.popleft())
                    if nxt is not None:
                        if c == 0:
                            _emit_corr()
                            nxt_st = _start_load(g + 1)
                            nxt.append(nxt_st)
                        # spread next pair's RoPE chunks across this pair's
                        # chunks so DVE bursts stay short
                        _rope_chunk(nxt[0], c)
                    elif c == 0:
                        _emit_corr()
                while pending:
                    _emit_av(pending.popleft())

            def _pairs_body():
                st = _start_load(0, chunked=True)
                for ch in range(NCH):
                    _rope_chunk(st, ch)
                for g in range(PAIRS):
                    nxt = [] if g + 1 < PAIRS else None
                    _one_pair(g, st, nxt)
                    st = nxt[0] if nxt else None

            if reps == 1:
                _pairs_body()
            else:
                # unroll several reps per hardware-loop iteration: the
                # For_i back edge is an all-engine barrier, so copy
                # boundaries inside the body overlap while only the outer
                # edge pays the drain/refill cost
                u = 4 if reps % 4 == 0 else (2 if reps % 2 == 0 else 1)
                with tc.For_i(0, reps // u, 1,
                              hint_engines=(mybir.EngineType.PE,
                                            mybir.EngineType.Activation,
                                            mybir.EngineType.SP,
                                            mybir.EngineType.DVE,
                                            mybir.EngineType.Pool)):
                    for _ in range(u):
                        _pairs_body()

    nc.compile()
    return nc


_PROGRAMS = {}


def _get_program(reps=1):
    if reps not in _PROGRAMS:
        _PROGRAMS[reps] = _build_program(reps)
    return _PROGRAMS[reps]


def _shard_inputs(Q, K, V):
    import ml_dtypes
    BF = ml_dtypes.bfloat16
    consts = _host_consts()
    d = np.arange(HS)
    perm = np.concatenate([d[0::2], d[1::2]])  # deinterleave head dim
    rot = np.concatenate([np.arange(64, 128), np.arange(0, 64)])

    in_maps = []
    for core in range(NCORES):
        qkT = np.empty((PAIRS, 2, HS, T), BF)
        v = np.empty((PAIRS, NKT, JT, HS), BF)
        vT = np.empty((PAIRS, HS, T), BF)
        for slot in range(PAIRS):
            g = core * PAIRS + slot
            b, h = divmod(g, NH)
            cols = h * HS + np.arange(HS)
            qkT[slot, 0] = Q[b][:, cols[perm]].T.astype(BF)
            qkT[slot, 1] = K[b][:, cols[perm]].T.astype(BF)
            v[slot] = V[b][:, cols].reshape(JT, NKT, HS).transpose(1, 0, 2).astype(BF)
            vT[slot] = V[b][:, cols].T.astype(BF)
        in_maps.append({
            "qkT": np.ascontiguousarray(qkT),
            "qkR": np.ascontiguousarray(qkT[:, :, rot, :]),
            "v": np.ascontiguousarray(v),
            "rope_c": consts["rope_c"],
            "rope_s": consts["rope_s"],
            "dmin": consts["dmin"],
            "vT": np.ascontiguousarray(vT),
        })
    return in_maps


def _gather_outputs(per_core_outT):
    out = np.empty((B, T, C_EMB), np.float32)
    for core in range(NCORES):
        outT = per_core_outT[core]  # [PAIRS, HS, T]
        for slot in range(PAIRS):
            g = core * PAIRS + slot
            b, h = divmod(g, NH)
            out[b, :, h * HS:(h + 1) * HS] = outT[slot].T
    return out


def kernel(Q, K, V):
    from concourse.bass_utils import run_bass_kernel_spmd

    Q = np.asarray(Q, dtype=np.float32)
    K = np.asarray(K, dtype=np.float32)
    V = np.asarray(V, dtype=np.float32)

    nc = _get_program()
    in_maps = _shard_inputs(Q, K, V)
    res = run_bass_kernel_spmd(nc, in_maps, core_ids=list(range(NCORES)))
    return _gather_outputs([res.results[c]["outT"] for c in range(NCORES)])
